# revision 12
# baseline (speedup 1.0000x reference)
"""LoRA kernel for TRN2: y = (x @ A) @ B * scale, data-parallel over 8 cores.

Reference materializes W = (A@B)*SCALE [4096,4096] then x@W (~275 GFLOP).
Mathematically identical low-rank evaluation: u = x@(A*SCALE) [rows,8],
y = u@B — ~2 GFLOP, memory-bound.

v2 plan (per core; rows sharded 8192/8 = 1024, A/B replicated, all bf16):
  x is pre-transposed + cast to bf16 on host: xt [4096, 1024].
  Rows split in RBLK blocks of 512 so block b+1's loads overlap block b's
  y-phase stores (HBM is the floor: 8.4 MB in + 8.4 MB out ≈ 47 us).
  Per block:
    u^T [8, 512] accumulated in PSUM over 32 feature chunks:
      matmul(lhsT=A_kc [128,8] bf16, rhs=xt chunk [128,512] bf16)
    copy+cast u^T -> SBUF bf16
    y row tiles (4x128): matmul(lhsT=u^T slice [8,128], rhs=B [8,512])
      -> PSUM fp32, copy+cast -> SBUF bf16 (split across DVE+ACT), DMA out.
  Host upcasts y bf16 -> fp32. End-to-end rel err ~6e-3 (tol 2e-2).
"""

import contextlib
import os

import numpy as np
import ml_dtypes

os.environ.setdefault("MYCRO_LOCAL_CACHE", "1")

import concourse.bacc as bacc
import concourse.mybir as mybir
import concourse.tile as tile
from concourse.bass_utils import run_bass_kernel_spmd

F32 = mybir.dt.float32
BF16 = mybir.dt.bfloat16
NP_BF16 = ml_dtypes.bfloat16

N_CORES = 8
BATCH, SEQ, D = 4, 2048, 4096
RANK = 8
SCALE = 16 / 8
ROWS = BATCH * SEQ            # 8192
R_CORE = ROWS // N_CORES      # 1024 rows per core
P = 128                       # partitions
KC = D // P                   # 32 feature chunks of 128
RBLK = 2                      # row blocks per core
RB_ROWS = R_CORE // RBLK      # 512 rows per block
GK = 4                        # feature chunks per xt load group
G = KC // GK                  # 8 load groups per block

_NC_CACHE = {}


def build_v2(repeat=1, hwloop=False, rblk=RBLK, store_eng="scalar", gk=GK):
    nc = bacc.Bacc("TRN2", target_bir_lowering=False, debug=False)
    rb_rows = R_CORE // rblk
    g_cnt = KC // gk

    xt_d = nc.dram_tensor("xt", [D, R_CORE], BF16, kind="ExternalInput")
    a_d = nc.dram_tensor("A", [D, RANK], BF16, kind="ExternalInput")
    b_d = nc.dram_tensor("B", [RANK, D], BF16, kind="ExternalInput")
    y_d = nc.dram_tensor("y", [R_CORE, D], BF16, kind="ExternalOutput")

    with tile.TileContext(nc) as tc:
        with (
            tc.tile_pool(name="const", bufs=1) as cpool,
            tc.tile_pool(name="xtp", bufs=6) as xtp,
            tc.tile_pool(name="utp", bufs=2) as utp,
            tc.tile_pool(name="ypool", bufs=3) as ypool,
            tc.tile_pool(name="ps_u", bufs=(2 if rblk >= 2 else 1),
                         space="PSUM") as ps_u,
            tc.tile_pool(name="ps_y", bufs=3, space="PSUM") as ps_y,
        ):
            a_sb = cpool.tile([P, KC, RANK], BF16)
            nc.sync.dma_start(a_sb[:], a_d[:, :].rearrange("(kc p) r -> p kc r", p=P))
            b_sb = cpool.tile([RANK, D], BF16)
            nc.sync.dma_start(b_sb[:], b_d[:, :])

            if hwloop and repeat > 1:
                loop_cm = tc.For_i(0, repeat, 1)
                n_unroll = 1
            else:
                loop_cm = contextlib.nullcontext()
                n_unroll = repeat
            with loop_cm:
              for rep in range(n_unroll):
                for blk in range(rblk):
                    col0 = blk * rb_rows
                    u_ps = ps_u.tile([RANK, rb_rows], F32, tag="u")
                    for g in range(g_cnt):
                        xt_sb = xtp.tile([P, gk, rb_rows], BF16, tag="xt")
                        nc.sync.dma_start(
                            xt_sb[:],
                            xt_d[g * gk * P:(g + 1) * gk * P,
                                 col0:col0 + rb_rows]
                            .rearrange("(kc p) r -> p kc r", p=P),
                        )
                        for j in range(gk):
                            kc = g * gk + j
                            for rb2 in range(rb_rows // 512):
                                nc.tensor.matmul(
                                    u_ps[:, rb2 * 512:(rb2 + 1) * 512],
                                    a_sb[:, kc, :],
                                    xt_sb[:, j, rb2 * 512:(rb2 + 1) * 512],
                                    start=(kc == 0),
                                    stop=(kc == KC - 1),
                                )
                    ut_sb = utp.tile([RANK, rb_rows], BF16, tag="ut")
                    nc.vector.tensor_copy(ut_sb[:], u_ps[:])

                    for rt in range(rb_rows // P):
                        y_sb = ypool.tile([P, D], BF16, tag="y")
                        for jj in range(4):
                            y_ps = ps_y.tile([P, 1024], F32, tag="y_ps")
                            for j2 in range(2):
                                nc.tensor.matmul(
                                    y_ps[:, j2 * 512:(j2 + 1) * 512],
                                    ut_sb[:, rt * P:(rt + 1) * P],
                                    b_sb[:, jj * 1024 + j2 * 512:
                                         jj * 1024 + (j2 + 1) * 512],
                                )
                            if jj % 2 == 0:
                                nc.vector.tensor_copy(
                                    y_sb[:, jj * 1024:(jj + 1) * 1024], y_ps[:])
                            else:
                                nc.scalar.copy(
                                    y_sb[:, jj * 1024:(jj + 1) * 1024], y_ps[:])
                        row0 = col0 + rt * P
                        if store_eng == "scalar":
                            nc.scalar.dma_start(y_d[row0:row0 + P, :], y_sb[:])
                        else:
                            nc.sync.dma_start(y_d[row0:row0 + P, :], y_sb[:])

    nc.compile()
    return nc


def build_v3(repeat=1, hwloop=False, mode="pad"):
    """v3: phase-separated DMA directions + PE-friendly MM shapes.

    Empirical env model (microbench): interleaved ld/st DMA collapses to
    ~86 GB/s while batched directions stream ~400 GB/s each way; matmuls
    cost ~380 ns (m=128, same lhsT, same-bank distance >=8), +~255 ns on
    lhsT switch, ~660-760 ns for m=8-partition outputs regardless.

    Per rep (1024 rows/core):
      u-phase (loads only): 8 zero-padded accumulate chains, one per
        (kc%4, rowhalf), across all 8 PSUM banks -> same-bank distance 8.
        lhsT = A_kc zero-padded to [128,128] so m=128 (mode="pad").
      reduce: ut[rh] = sum of 4 partials' rows 0:8 (DVE adds, cast bf16).
      y-phase (stores only): 64 MMs rotating the same 4 [128,1024] PSUM
        tiles (distance 8), lhsT switches once per 8 MMs; copies split
        DVE/ACT; stores on the scalar ring.
    """
    nc = bacc.Bacc("TRN2", target_bir_lowering=False, debug=False)

    xt_d = nc.dram_tensor("xt", [D, R_CORE], BF16, kind="ExternalInput")
    if mode == "pad":
        a_d = nc.dram_tensor("Apad", [P, KC, P], BF16, kind="ExternalInput")
        a_m = P
    else:
        a_d = nc.dram_tensor("Apad", [P, KC, RANK], BF16, kind="ExternalInput")
        a_m = RANK
    b_d = nc.dram_tensor("B", [RANK, D], BF16, kind="ExternalInput")
    y_d = nc.dram_tensor("y", [R_CORE, D], BF16, kind="ExternalOutput")

    with tile.TileContext(nc) as tc:
        with (
            tc.tile_pool(name="const", bufs=1) as cpool,
            tc.tile_pool(name="xtp", bufs=4) as xtp,
            tc.tile_pool(name="utp", bufs=2) as utp,
            tc.tile_pool(name="red", bufs=2) as redp,
            tc.tile_pool(name="ypool", bufs=3) as ypool,
            tc.tile_pool(name="pp", bufs=4, space="PSUM") as pp,
        ):
            a_sb = cpool.tile([P, KC, a_m], BF16)
            nc.sync.dma_start(a_sb[:], a_d[:, :, :])
            b_sb = cpool.tile([RANK, D], BF16)
            nc.sync.dma_start(b_sb[:], b_d[:, :])

            if hwloop and repeat > 1:
                loop_cm = tc.For_i(0, repeat, 1)
                n_unroll = 1
            else:
                loop_cm = contextlib.nullcontext()
                n_unroll = repeat
            with loop_cm:
              for rep in range(n_unroll):
                # ---- u phase: loads + 8 accumulate chains over 8 banks
                up = [pp.tile([P, 1024], F32, tag="pp", name=f"up{i}")
                      for i in range(4)]
                for g in range(8):
                    xt_sb = xtp.tile([P, 4, R_CORE], BF16, tag="xt")
                    nc.sync.dma_start(
                        xt_sb[:],
                        xt_d[g * 4 * P:(g + 1) * 4 * P, :]
                        .rearrange("(kc p) r -> p kc r", p=P),
                    )
                    for j in range(4):
                        kc = g * 4 + j
                        c, t = kc % 4, kc // 4
                        for rh in range(2):
                            out = up[c][0:a_m, rh * 512:(rh + 1) * 512]
                            nc.tensor.matmul(
                                out,
                                a_sb[:, kc, :],
                                xt_sb[:, j, rh * 512:(rh + 1) * 512],
                                start=(t == 0),
                                stop=(t == 7),
                            )
                # ---- reduce partials -> ut [8, 1024] bf16
                ut_sb = utp.tile([RANK, R_CORE], BF16, tag="ut")
                for rh in range(2):
                    sl = slice(rh * 512, (rh + 1) * 512)
                    c0 = redp.tile([RANK, 512], F32, tag="c0")
                    c2 = redp.tile([RANK, 512], F32, tag="c2")
                    s1 = redp.tile([RANK, 512], F32, tag="s1")
                    s2 = redp.tile([RANK, 512], F32, tag="s2")
                    nc.vector.tensor_copy(c0[:], up[0][0:RANK, sl])
                    nc.vector.tensor_add(s1[:], up[1][0:RANK, sl], c0[:])
                    nc.vector.tensor_copy(c2[:], up[2][0:RANK, sl])
                    nc.vector.tensor_add(s2[:], up[3][0:RANK, sl], c2[:])
                    nc.vector.tensor_add(ut_sb[:, sl], s1[:], s2[:])

                # ---- y phase: 64 MMs rotating 4 [128,1024] tiles, stores
                for rt in range(8):
                    y_sb = ypool.tile([P, D], BF16, tag="y")
                    yt = [pp.tile([P, 1024], F32, tag="pp",
                                  name=f"yt{rt}_{i}") for i in range(4)]
                    for jj in range(8):
                        tj, h = jj // 2, jj % 2
                        nc.tensor.matmul(
                            yt[tj][:, h * 512:(h + 1) * 512],
                            ut_sb[:, rt * P:(rt + 1) * P],
                            b_sb[:, jj * 512:(jj + 1) * 512],
                        )
                    for tj in range(4):
                        dst = y_sb[:, tj * 1024:(tj + 1) * 1024]
                        if tj % 2 == 0:
                            nc.vector.tensor_copy(dst, yt[tj][:])
                        else:
                            nc.scalar.copy(dst, yt[tj][:])
                    row0 = rt * P
                    nc.scalar.dma_start(y_d[row0:row0 + P, :], y_sb[:])

    nc.compile()
    return nc


def _prep_apad(A, pad=True):
    af = np.asarray(A, dtype=np.float32) * np.float32(SCALE)
    m = P if pad else RANK
    apad = np.zeros((P, KC, m), dtype=NP_BF16)
    # apad[p, kc, 0:8] = A[kc*128 + p, :]
    apad[:, :, 0:RANK] = af.reshape(KC, P, RANK).transpose(1, 0, 2).astype(NP_BF16)
    return apad


def _prep_in_maps_v3(x, A, B, pad=True):
    xf = np.asarray(x, dtype=np.float32).reshape(ROWS, D)
    apad = _prep_apad(A, pad=pad)
    bf = np.asarray(B, dtype=np.float32).astype(NP_BF16)
    return [
        {
            "xt": np.ascontiguousarray(
                xf[c * R_CORE:(c + 1) * R_CORE].T.astype(NP_BF16)),
            "Apad": apad,
            "B": bf,
        }
        for c in range(N_CORES)
    ]


def _prep_in_maps(x, A, B):
    xf = np.asarray(x, dtype=np.float32).reshape(ROWS, D)
    af = (np.asarray(A, dtype=np.float32) * np.float32(SCALE)).astype(NP_BF16)
    bf = np.asarray(B, dtype=np.float32).astype(NP_BF16)
    return [
        {
            "xt": np.ascontiguousarray(
                xf[c * R_CORE:(c + 1) * R_CORE].T.astype(NP_BF16)),
            "A": af,
            "B": bf,
        }
        for c in range(N_CORES)
    ]


def get_nc():
    if "v2" not in _NC_CACHE:
        _NC_CACHE["v2"] = build_v2()
    return _NC_CACHE["v2"]


def kernel(x, A, B, _nc=None, **run_kwargs):
    nc = _nc if _nc is not None else get_nc()
    in_maps = _prep_in_maps(x, A, B)
    res = run_bass_kernel_spmd(nc, in_maps, core_ids=list(range(N_CORES)), **run_kwargs)
    y = np.concatenate([r["y"] for r in res.results], axis=0)
    out = y.astype(np.float32).reshape(BATCH, SEQ, D)
    if run_kwargs:
        return out, res
    return out


# revision 17
# speedup vs baseline: 1.2724x; 1.2724x over previous
"""LoRA kernel for TRN2: y = (x @ A) @ B * scale, data-parallel over 8 cores.

Reference materializes W = (A@B)*SCALE [4096,4096] then x@W (~275 GFLOP).
Mathematically identical low-rank evaluation: u = x@(A*SCALE) [rows,8],
y = u@B — ~2 GFLOP, memory-bound.

v2 plan (per core; rows sharded 8192/8 = 1024, A/B replicated, all bf16):
  x is pre-transposed + cast to bf16 on host: xt [4096, 1024].
  Rows split in RBLK blocks of 512 so block b+1's loads overlap block b's
  y-phase stores (HBM is the floor: 8.4 MB in + 8.4 MB out ≈ 47 us).
  Per block:
    u^T [8, 512] accumulated in PSUM over 32 feature chunks:
      matmul(lhsT=A_kc [128,8] bf16, rhs=xt chunk [128,512] bf16)
    copy+cast u^T -> SBUF bf16
    y row tiles (4x128): matmul(lhsT=u^T slice [8,128], rhs=B [8,512])
      -> PSUM fp32, copy+cast -> SBUF bf16 (split across DVE+ACT), DMA out.
  Host upcasts y bf16 -> fp32. End-to-end rel err ~6e-3 (tol 2e-2).
"""

import contextlib
import os

import numpy as np
import ml_dtypes

os.environ.setdefault("MYCRO_LOCAL_CACHE", "1")

import concourse.bacc as bacc
import concourse.mybir as mybir
import concourse.tile as tile
from concourse.bass_utils import run_bass_kernel_spmd

F32 = mybir.dt.float32
BF16 = mybir.dt.bfloat16
NP_BF16 = ml_dtypes.bfloat16

N_CORES = 8
BATCH, SEQ, D = 4, 2048, 4096
RANK = 8
SCALE = 16 / 8
ROWS = BATCH * SEQ            # 8192
R_CORE = ROWS // N_CORES      # 1024 rows per core
P = 128                       # partitions
KC = D // P                   # 32 feature chunks of 128
RBLK = 2                      # row blocks per core
RB_ROWS = R_CORE // RBLK      # 512 rows per block
GK = 4                        # feature chunks per xt load group
G = KC // GK                  # 8 load groups per block

_NC_CACHE = {}


def build_v2(repeat=1, hwloop=False, rblk=RBLK, store_eng="scalar", gk=GK,
             uchains=1):
    nc = bacc.Bacc("TRN2", target_bir_lowering=False, debug=False)
    rb_rows = R_CORE // rblk
    g_cnt = KC // gk

    xt_d = nc.dram_tensor("xt", [D, R_CORE], BF16, kind="ExternalInput")
    a_d = nc.dram_tensor("A", [D, RANK], BF16, kind="ExternalInput")
    b_d = nc.dram_tensor("B", [RANK, D], BF16, kind="ExternalInput")
    y_d = nc.dram_tensor("y", [R_CORE, D], BF16, kind="ExternalOutput")

    with tile.TileContext(nc) as tc:
        with (
            tc.tile_pool(name="const", bufs=1) as cpool,
            tc.tile_pool(name="xtp", bufs=6) as xtp,
            tc.tile_pool(name="utp", bufs=2) as utp,
            tc.tile_pool(name="red", bufs=2) as redp,
            tc.tile_pool(name="ypool", bufs=3) as ypool,
            tc.tile_pool(name="ps_u",
                         bufs=(1 if uchains > 1 else (2 if rblk >= 2 else 1)),
                         space="PSUM") as ps_u,
            tc.tile_pool(name="ps_y", bufs=3, space="PSUM") as ps_y,
        ):
            a_sb = cpool.tile([P, KC, RANK], BF16)
            nc.sync.dma_start(a_sb[:], a_d[:, :].rearrange("(kc p) r -> p kc r", p=P))
            b_sb = cpool.tile([RANK, D], BF16)
            nc.sync.dma_start(b_sb[:], b_d[:, :])

            if hwloop and repeat > 1:
                loop_cm = tc.For_i(0, repeat, 1)
                n_unroll = 1
            else:
                loop_cm = contextlib.nullcontext()
                n_unroll = repeat
            with loop_cm:
              for rep in range(n_unroll):
                for blk in range(rblk):
                    col0 = blk * rb_rows
                    if uchains == 1:
                        u_list = [ps_u.tile([RANK, rb_rows], F32, tag="u",
                                            name=f"u{blk}")]
                    else:
                        u_list = [ps_u.tile([RANK, rb_rows], F32, tag=f"u{c}",
                                            name=f"u{blk}_{c}")
                                  for c in range(uchains)]
                    for g in range(g_cnt):
                        xt_sb = xtp.tile([P, gk, rb_rows], BF16, tag="xt")
                        nc.sync.dma_start(
                            xt_sb[:],
                            xt_d[g * gk * P:(g + 1) * gk * P,
                                 col0:col0 + rb_rows]
                            .rearrange("(kc p) r -> p kc r", p=P),
                        )
                        for j in range(gk):
                            kc = g * gk + j
                            u_ps = u_list[kc % uchains]
                            for rb2 in range(rb_rows // 512):
                                nc.tensor.matmul(
                                    u_ps[:, rb2 * 512:(rb2 + 1) * 512],
                                    a_sb[:, kc, :],
                                    xt_sb[:, j, rb2 * 512:(rb2 + 1) * 512],
                                    start=(kc < uchains),
                                    stop=(kc >= KC - uchains),
                                )
                    ut_sb = utp.tile([RANK, rb_rows], BF16, tag="ut")
                    if uchains == 1:
                        nc.vector.tensor_copy(ut_sb[:], u_list[0][:])
                    else:
                        c0 = redp.tile([RANK, rb_rows], F32, tag="c0")
                        nc.vector.tensor_copy(c0[:], u_list[0][:])
                        acc = c0
                        for c in range(1, uchains - 1):
                            nx = redp.tile([RANK, rb_rows], F32, tag=f"a{c}",
                                           name=f"acc{blk}_{c}")
                            nc.vector.tensor_add(nx[:], u_list[c][:], acc[:])
                            acc = nx
                        nc.vector.tensor_add(ut_sb[:], u_list[uchains - 1][:],
                                             acc[:])

                    for rt in range(rb_rows // P):
                        y_sb = ypool.tile([P, D], BF16, tag="y")
                        for jj in range(4):
                            y_ps = ps_y.tile([P, 1024], F32, tag="y_ps")
                            for j2 in range(2):
                                nc.tensor.matmul(
                                    y_ps[:, j2 * 512:(j2 + 1) * 512],
                                    ut_sb[:, rt * P:(rt + 1) * P],
                                    b_sb[:, jj * 1024 + j2 * 512:
                                         jj * 1024 + (j2 + 1) * 512],
                                )
                            if jj % 2 == 0:
                                nc.vector.tensor_copy(
                                    y_sb[:, jj * 1024:(jj + 1) * 1024], y_ps[:])
                            else:
                                nc.scalar.copy(
                                    y_sb[:, jj * 1024:(jj + 1) * 1024], y_ps[:])
                        row0 = col0 + rt * P
                        if store_eng == "scalar":
                            nc.scalar.dma_start(y_d[row0:row0 + P, :], y_sb[:])
                        else:
                            nc.sync.dma_start(y_d[row0:row0 + P, :], y_sb[:])

    nc.compile()
    return nc


def build_v3(repeat=1, hwloop=False, mode="pad"):
    """v3: phase-separated DMA directions + PE-friendly MM shapes.

    Empirical env model (microbench): interleaved ld/st DMA collapses to
    ~86 GB/s while batched directions stream ~400 GB/s each way; matmuls
    cost ~380 ns (m=128, same lhsT, same-bank distance >=8), +~255 ns on
    lhsT switch, ~660-760 ns for m=8-partition outputs regardless.

    Per rep (1024 rows/core):
      u-phase (loads only): 8 zero-padded accumulate chains, one per
        (kc%4, rowhalf), across all 8 PSUM banks -> same-bank distance 8.
        lhsT = A_kc zero-padded to [128,128] so m=128 (mode="pad").
      reduce: ut[rh] = sum of 4 partials' rows 0:8 (DVE adds, cast bf16).
      y-phase (stores only): 64 MMs rotating the same 4 [128,1024] PSUM
        tiles (distance 8), lhsT switches once per 8 MMs; copies split
        DVE/ACT; stores on the scalar ring.
    """
    nc = bacc.Bacc("TRN2", target_bir_lowering=False, debug=False)

    xt_d = nc.dram_tensor("xt", [D, R_CORE], BF16, kind="ExternalInput")
    if mode == "pad":
        a_d = nc.dram_tensor("Apad", [P, KC, P], BF16, kind="ExternalInput")
        a_m = P
    else:
        a_d = nc.dram_tensor("Apad", [P, KC, RANK], BF16, kind="ExternalInput")
        a_m = RANK
    b_d = nc.dram_tensor("B", [RANK, D], BF16, kind="ExternalInput")
    y_d = nc.dram_tensor("y", [R_CORE, D], BF16, kind="ExternalOutput")

    with tile.TileContext(nc) as tc:
        with (
            tc.tile_pool(name="const", bufs=1) as cpool,
            tc.tile_pool(name="xtp", bufs=4) as xtp,
            tc.tile_pool(name="utp", bufs=2) as utp,
            tc.tile_pool(name="red", bufs=2) as redp,
            tc.tile_pool(name="ypool", bufs=3) as ypool,
            tc.tile_pool(name="pp", bufs=4, space="PSUM") as pp,
        ):
            a_sb = cpool.tile([P, KC, a_m], BF16)
            nc.sync.dma_start(a_sb[:], a_d[:, :, :])
            b_sb = cpool.tile([RANK, D], BF16)
            nc.sync.dma_start(b_sb[:], b_d[:, :])

            if hwloop and repeat > 1:
                loop_cm = tc.For_i(0, repeat, 1)
                n_unroll = 1
            else:
                loop_cm = contextlib.nullcontext()
                n_unroll = repeat
            with loop_cm:
              for rep in range(n_unroll):
                # ---- u phase: loads + 8 accumulate chains over 8 banks
                up = [pp.tile([P, 1024], F32, tag="pp", name=f"up{i}")
                      for i in range(4)]
                for g in range(8):
                    xt_sb = xtp.tile([P, 4, R_CORE], BF16, tag="xt")
                    nc.sync.dma_start(
                        xt_sb[:],
                        xt_d[g * 4 * P:(g + 1) * 4 * P, :]
                        .rearrange("(kc p) r -> p kc r", p=P),
                    )
                    for j in range(4):
                        kc = g * 4 + j
                        c, t = kc % 4, kc // 4
                        for rh in range(2):
                            out = up[c][0:a_m, rh * 512:(rh + 1) * 512]
                            nc.tensor.matmul(
                                out,
                                a_sb[:, kc, :],
                                xt_sb[:, j, rh * 512:(rh + 1) * 512],
                                start=(t == 0),
                                stop=(t == 7),
                            )
                # ---- reduce partials -> ut [8, 1024] bf16
                ut_sb = utp.tile([RANK, R_CORE], BF16, tag="ut")
                for rh in range(2):
                    sl = slice(rh * 512, (rh + 1) * 512)
                    c0 = redp.tile([RANK, 512], F32, tag="c0")
                    c2 = redp.tile([RANK, 512], F32, tag="c2")
                    s1 = redp.tile([RANK, 512], F32, tag="s1")
                    s2 = redp.tile([RANK, 512], F32, tag="s2")
                    nc.vector.tensor_copy(c0[:], up[0][0:RANK, sl])
                    nc.vector.tensor_add(s1[:], up[1][0:RANK, sl], c0[:])
                    nc.vector.tensor_copy(c2[:], up[2][0:RANK, sl])
                    nc.vector.tensor_add(s2[:], up[3][0:RANK, sl], c2[:])
                    nc.vector.tensor_add(ut_sb[:, sl], s1[:], s2[:])

                # ---- y phase: 64 MMs rotating 4 [128,1024] tiles, stores
                for rt in range(8):
                    y_sb = ypool.tile([P, D], BF16, tag="y")
                    yt = [pp.tile([P, 1024], F32, tag="pp",
                                  name=f"yt{rt}_{i}") for i in range(4)]
                    for jj in range(8):
                        tj, h = jj // 2, jj % 2
                        nc.tensor.matmul(
                            yt[tj][:, h * 512:(h + 1) * 512],
                            ut_sb[:, rt * P:(rt + 1) * P],
                            b_sb[:, jj * 512:(jj + 1) * 512],
                        )
                    for tj in range(4):
                        dst = y_sb[:, tj * 1024:(tj + 1) * 1024]
                        if tj % 2 == 0:
                            nc.vector.tensor_copy(dst, yt[tj][:])
                        else:
                            nc.scalar.copy(dst, yt[tj][:])
                    row0 = rt * P
                    nc.scalar.dma_start(y_d[row0:row0 + P, :], y_sb[:])

    nc.compile()
    return nc


def _prep_apad(A, pad=True):
    af = np.asarray(A, dtype=np.float32) * np.float32(SCALE)
    m = P if pad else RANK
    apad = np.zeros((P, KC, m), dtype=NP_BF16)
    # apad[p, kc, 0:8] = A[kc*128 + p, :]
    apad[:, :, 0:RANK] = af.reshape(KC, P, RANK).transpose(1, 0, 2).astype(NP_BF16)
    return apad


def _prep_in_maps_v3(x, A, B, pad=True):
    xf = np.asarray(x, dtype=np.float32).reshape(ROWS, D)
    apad = _prep_apad(A, pad=pad)
    bf = np.asarray(B, dtype=np.float32).astype(NP_BF16)
    return [
        {
            "xt": np.ascontiguousarray(
                xf[c * R_CORE:(c + 1) * R_CORE].T.astype(NP_BF16)),
            "Apad": apad,
            "B": bf,
        }
        for c in range(N_CORES)
    ]


def _prep_in_maps(x, A, B):
    xf = np.asarray(x, dtype=np.float32).reshape(ROWS, D)
    af = (np.asarray(A, dtype=np.float32) * np.float32(SCALE)).astype(NP_BF16)
    bf = np.asarray(B, dtype=np.float32).astype(NP_BF16)
    return [
        {
            "xt": np.ascontiguousarray(
                xf[c * R_CORE:(c + 1) * R_CORE].T.astype(NP_BF16)),
            "A": af,
            "B": bf,
        }
        for c in range(N_CORES)
    ]


def get_nc():
    if "v2" not in _NC_CACHE:
        _NC_CACHE["v2"] = build_v2()
    return _NC_CACHE["v2"]


def kernel(x, A, B, _nc=None, **run_kwargs):
    nc = _nc if _nc is not None else get_nc()
    in_maps = _prep_in_maps(x, A, B)
    res = run_bass_kernel_spmd(nc, in_maps, core_ids=list(range(N_CORES)), **run_kwargs)
    y = np.concatenate([r["y"] for r in res.results], axis=0)
    out = y.astype(np.float32).reshape(BATCH, SEQ, D)
    if run_kwargs:
        return out, res
    return out
